# revision 6
# baseline (speedup 1.0000x reference)
"""TRN2 Bass kernel for GPT-style causal self-attention with RoPE (bf16).

Reference (B=2, S=2048, D=1024, H=16, dk=64):
  qkv = hidden @ c_attn_w + c_attn_b; rope(q), rope(k) via position_ids;
  out = softmax(causal(q k^T / 8)) v, merged heads, @ c_proj_w + c_proj_b.

Sharding across 8 NeuronCores: core c = 4*b + g handles batch b and head
group g (4 heads = 256 dims). Each core computes its full S x S attention
for its heads and a row-sliced c_proj partial; the host sums the 4
partials per batch.

Device pipeline per core (all matmuls bf16, fp32 PSUM accumulate):
  1. QKV weight-stationary: qkvT[do, s] = Wqkv_chunk^T @ hT directly in
     transposed layout (no PE transposes for q/k). Rope applied in the
     transposed layout: partition-swap via SBUF-SBUF DMAs (gpsimd queue)
     + 3 DVE ops against host-precomputed cosT/sinT tables. V transposed
     back to natural [s, d] via PE transposes with a ones column.
  2. Per head-pair, per 512-wide q chunk: scores^T via K=64 matmul pairs
     (two heads in PE quadrants); exp on ScalarE (scale=1/8, its only
     job); causal diagonal mask (0/1) on GPSIMD post-exp; PV accumulates
     [v|1]^T P^T (row 64 = softmax denominators). PSUM is evicted to
     SBUF by DVE immediately; recip + partition_broadcast + multiply run
     off the PE critical path. Emission is staggered (scores of the next
     chunk precede PV of the current one) so the in-order PE queue
     always has ready work while exp drains.
  3. Projection interleaved per 512-q chunk: projT = Wp^T @ attnT,
     DVE eviction (bias via ScalarE only in the with_bias variant),
     bf16 DMA out.
Output per core: outT [1024, 2048] bf16 partial; host sums per batch.
"""

from contextlib import ExitStack

import numpy as np
import ml_dtypes

import concourse.bacc as bacc
import concourse.tile as tile
import concourse.mybir as mybir
from concourse.bass_utils import run_bass_kernel_spmd

f32 = mybir.dt.float32
bf16 = mybir.dt.bfloat16
AF = mybir.ActivationFunctionType
ALU = mybir.AluOpType

S = 2048
D = 1024
HD = 256           # head dims per core (4 heads x 64)
SB = S // 128      # 16
KC = D // 128      # 8
NCH = S // 512     # 4
BF = ml_dtypes.bfloat16


def build_attention_nc(with_bias=False, num_devices=8):
    nc = bacc.Bacc("TRN2", target_bir_lowering=False, debug=False,
                   num_devices=num_devices)

    hT_d = nc.dram_tensor("hT", [D, S], bf16, kind="ExternalInput")
    wqkv_d = nc.dram_tensor("wqkv", [D, 768], bf16, kind="ExternalInput")
    cosT_d = nc.dram_tensor("cosT", [128, S], bf16, kind="ExternalInput")
    sinT_d = nc.dram_tensor("sinT", [128, S], bf16, kind="ExternalInput")
    wp_d = nc.dram_tensor("wp", [HD, D], bf16, kind="ExternalInput")
    bp_d = nc.dram_tensor("bp", [128, 8], f32, kind="ExternalInput")
    mask01_d = nc.dram_tensor("mask01", [128, 128], bf16, kind="ExternalInput")
    ones64_d = nc.dram_tensor("ones64", [128, 64], bf16, kind="ExternalInput")
    ident_d = nc.dram_tensor("ident", [128, 128], bf16, kind="ExternalInput")
    if with_bias:
        bqkv_d = nc.dram_tensor("bqkv", [1, 768], bf16, kind="ExternalInput")
        onesrow_d = nc.dram_tensor("ones_row", [1, 512], bf16,
                                   kind="ExternalInput")
    outT_d = nc.dram_tensor("outT", [D, S], bf16, kind="ExternalOutput")

    with tile.TileContext(nc) as tc, ExitStack() as top:
        const = top.enter_context(tc.tile_pool(name="const", bufs=1))
        ident = const.tile([128, 128], bf16, tag="ident")
        nc.sync.dma_start(ident[:], ident_d.ap())
        mask01 = const.tile([128, 128], bf16, tag="mask01")
        nc.sync.dma_start(mask01[:], mask01_d.ap())
        bp_sb = const.tile([128, 8], f32, tag="bp")
        nc.sync.dma_start(bp_sb[:], bp_d.ap())
        if with_bias:
            bqkv_sb = const.tile([1, 768], bf16, tag="bqkv")
            nc.sync.dma_start(bqkv_sb[:], bqkv_d.ap())
            ones_row = const.tile([1, 512], bf16, tag="ones_row")
            nc.sync.dma_start(ones_row[:], onesrow_d.ap())

        persist = top.enter_context(tc.tile_pool(name="persist", bufs=1))
        qT = [persist.tile([128, S], bf16, tag=f"qT{hp}", name=f"qT{hp}")
              for hp in range(2)]
        kT = [persist.tile([128, S], bf16, tag=f"kT{hp}", name=f"kT{hp}")
              for hp in range(2)]
        v_sb = persist.tile([128, SB, 4, 65], bf16, tag="v")
        ones64 = const.tile([128, 64], bf16, tag="ones64")
        nc.sync.dma_start(ones64[:], ones64_d.ap())
        nc.scalar.copy(v_sb[:, :, :, 64],
                       ones64[:].rearrange("p (a b) -> p a b", a=SB))
        wp_sb = persist.tile([128, 2, D], bf16, tag="wp")
        for kc2 in range(2):
            nc.sync.dma_start(wp_sb[:, kc2, :],
                              wp_d.ap()[kc2 * 128:(kc2 + 1) * 128, :])
        attnT = [persist.tile([128, S], bf16, tag=f"attnT{hp}",
                              name=f"attnT{hp}") for hp in range(2)]
        cosT = persist.tile([128, S], bf16, tag="cosT")
        sinT = persist.tile([128, S], bf16, tag="sinT")
        nc.sync.dma_start(cosT[:], cosT_d.ap())
        nc.sync.dma_start(sinT[:], sinT_d.ap())

        # ============ stage 1: QKV + rope (transposed layout) ============
        with ExitStack() as st1, nc.named_scope("qkv"):
            hT_pool = st1.enter_context(tc.tile_pool(name="hT", bufs=1))
            w_pool = st1.enter_context(tc.tile_pool(name="w", bufs=1))
            vT_pool = st1.enter_context(tc.tile_pool(name="vT", bufs=1))
            qkv_ps = st1.enter_context(
                tc.tile_pool(name="qkv_ps", bufs=3, space="PSUM"))
            tr_ps = st1.enter_context(
                tc.tile_pool(name="tr_ps", bufs=2, space="PSUM"))
            rope_pool = st1.enter_context(tc.tile_pool(name="rope", bufs=2))

            w_sb = [w_pool.tile([128, 768], bf16, tag=f"w{kc}", name=f"w{kc}")
                    for kc in range(KC)]
            hT_sb = [hT_pool.tile([128, S], bf16, tag=f"hT{kc}",
                                  name=f"hT{kc}") for kc in range(KC)]
            for kc in range(KC):
                nc.sync.dma_start(w_sb[kc][:],
                                  wqkv_d.ap()[kc * 128:(kc + 1) * 128, :])
                nc.sync.dma_start(hT_sb[kc][:, 0:512],
                                  hT_d.ap()[kc * 128:(kc + 1) * 128, 0:512])
            for sblk in range(1, NCH):
                sl = slice(sblk * 512, (sblk + 1) * 512)
                for kc in range(KC):
                    nc.sync.dma_start(hT_sb[kc][:, sl],
                                      hT_d.ap()[kc * 128:(kc + 1) * 128, sl])

            vT_sb = [vT_pool.tile([128, S], bf16, tag=f"vT{t}", name=f"vT{t}")
                     for t in range(2)]

            # v chunks first (do 4,5), then q/k: q hp0=0, k hp0=2, q hp1=1,
            # k hp1=3
            for do in (4, 5, 0, 2, 1, 3):
                if do < 4:
                    qraw = rope_pool.tile([128, S], bf16, tag="qraw")
                for sblk in range(NCH):
                    sl = slice(sblk * 512, (sblk + 1) * 512)
                    qkv_p = qkv_ps.tile([128, 512], f32, tag="qkv_p")
                    for kc in range(KC):
                        nc.tensor.matmul(
                            qkv_p[:], w_sb[kc][:, do * 128:(do + 1) * 128],
                            hT_sb[kc][:, sl], start=(kc == 0),
                            stop=(kc == KC - 1 and not with_bias))
                    if with_bias:
                        nc.tensor.matmul(
                            qkv_p[:],
                            bqkv_sb[:, do * 128:(do + 1) * 128],
                            ones_row[:], start=False, stop=True)
                    if do >= 4:
                        nc.vector.tensor_copy(vT_sb[do - 4][:, sl], qkv_p[:])
                    else:
                        nc.vector.tensor_copy(qraw[:, sl], qkv_p[:])
                if do < 4:
                    dest = (qT if do in (0, 1) else kT)[do % 2]
                    qsw = rope_pool.tile([128, S], bf16, tag="qsw")
                    for blk in range(4):
                        src = (blk * 32 + 32) % 64 + 64 * (blk // 2)
                        nc.gpsimd.dma_start(qsw[blk * 32:blk * 32 + 32, :],
                                            qraw[src:src + 32, :])
                    qcos = rope_pool.tile([128, S], bf16, tag="qcos")
                    nc.vector.tensor_tensor(qcos[:], qraw[:], cosT[:],
                                            op=ALU.mult)
                    qsin = rope_pool.tile([128, S], bf16, tag="qsin")
                    nc.vector.tensor_tensor(qsin[:], qsw[:], sinT[:],
                                            op=ALU.mult)
                    nc.vector.tensor_tensor(dest[:], qcos[:], qsin[:],
                                            op=ALU.add)
                # after v chunks: transpose vT -> v natural layout
                if do == 5:
                    for t in range(2):
                        for sb in range(SB):
                            tp = tr_ps.tile([128, 128], bf16, tag="tp")
                            nc.tensor.matmul(
                                tp[:], vT_sb[t][:, sb * 128:(sb + 1) * 128],
                                ident[:], is_transpose=True,
                                start=True, stop=True)
                            nc.vector.tensor_copy(
                                v_sb[:, sb, 2 * t:2 * t + 2, 0:64],
                                tp[:].rearrange("p (h d) -> p h d", h=2))

        # ============ stages 2+3 interleaved ============
        st23 = top.enter_context(ExitStack())
        st_ps = st23.enter_context(
            tc.tile_pool(name="st_ps", bufs=2, space="PSUM"))
        out_ps = st23.enter_context(
            tc.tile_pool(name="out_ps", bufs=2, space="PSUM"))
        pj_ps = st23.enter_context(
            tc.tile_pool(name="pj_ps", bufs=2, space="PSUM"))
        pt_pool = st23.enter_context(tc.tile_pool(name="pt", bufs=17))
        u_pool = st23.enter_context(tc.tile_pool(name="u", bufs=4))
        nrm_pool = st23.enter_context(tc.tile_pool(name="nrm", bufs=4))
        pj_sb = st23.enter_context(tc.tile_pool(name="pj_sb", bufs=3))

        pts_map = {}

        def scores_chunk(c, hp):
            nkb = 4 * c + 4
            pts = []
            for kb in range(nkb):
                q0 = max(512 * c, 128 * kb)
                off = q0 - 512 * c
                st_p = st_ps.tile([128, 2, 512], f32, tag="st_p")
                for h2 in range(2):
                    nc.tensor.matmul(
                        st_p[:, h2, off:512],
                        kT[hp][h2 * 64:(h2 + 1) * 64,
                               kb * 128:(kb + 1) * 128],
                        qT[hp][h2 * 64:(h2 + 1) * 64, q0:512 * (c + 1)],
                        start=True, stop=True, tile_position=(h2 * 64, 0))
                pt = pt_pool.tile([128, 2, 512], bf16, tag="pt")
                nc.scalar.activation(pt[:, :, off:512], st_p[:, :, off:512],
                                     AF.Exp, scale=0.125)
                if 128 * kb >= 512 * c:
                    for h2 in range(2):
                        nc.gpsimd.tensor_mul(pt[:, h2, off:off + 128],
                                             pt[:, h2, off:off + 128],
                                             mask01[:])
                pts.append((kb, off, pt))
            pts_map[(c, hp)] = pts

        def pv_chunk(c, hp):
            nkb = 4 * c + 4
            pts = pts_map.pop((c, hp))
            for h2 in range(2):
                h = 2 * hp + h2
                o_p = out_ps.tile([128, 512], f32, tag="o_p")
                for (kb, off, pt) in pts:
                    nc.tensor.matmul(
                        o_p[0:65, off:512], v_sb[:, kb, h, :],
                        pt[:, h2, off:512],
                        start=(kb == 0), stop=(kb == nkb - 1))
                u = u_pool.tile([65, 512], f32, tag="u")
                nc.vector.tensor_copy(u[:], o_p[0:65, :])
                den0 = nrm_pool.tile([1, 512], f32, tag="den0")
                nc.sync.dma_start(den0[:], u[64:65, :])
                rcp0 = nrm_pool.tile([1, 512], f32, tag="rcp0")
                nc.vector.reciprocal_approx_fast(rcp0[:], den0[:])
                bc = nrm_pool.tile([64, 512], f32, tag="bc")
                nc.gpsimd.partition_broadcast(bc[:], rcp0[:])
                csl = slice(c * 512, (c + 1) * 512)
                if h2 == 0:
                    nc.vector.tensor_tensor(attnT[hp][0:64, csl],
                                            u[0:64, :], bc[:], op=ALU.mult)
                else:
                    aTo = u_pool.tile([64, 512], bf16, tag="aTo")
                    nc.vector.tensor_tensor(aTo[:], u[0:64, :], bc[:],
                                            op=ALU.mult)
                    nc.sync.dma_start(attnT[hp][64:128, csl], aTo[:])

        def proj_chunk(c):
            csl = slice(c * 512, (c + 1) * 512)
            for dd in range(8):
                pp = pj_ps.tile([128, 512], f32, tag="pp")
                for kc2 in range(2):
                    nc.tensor.matmul(
                        pp[:], wp_sb[:, kc2, dd * 128:(dd + 1) * 128],
                        attnT[kc2][:, csl],
                        start=(kc2 == 0), stop=(kc2 == 1))
                po = pj_sb.tile([128, 512], bf16, tag="po")
                if with_bias:
                    nc.scalar.activation(po[:], pp[:], AF.Identity,
                                         bias=bp_sb[:, dd:dd + 1])
                else:
                    nc.vector.tensor_copy(po[:], pp[:])
                nc.sync.dma_start(
                    outT_d.ap()[dd * 128:(dd + 1) * 128, csl], po[:])

        with nc.named_scope("attn"):
            # staggered emission: scores of the next chunk precede PV of
            # the current one so the in-order PE queue never waits on exp
            scores_chunk(0, 0)
            scores_chunk(0, 1)
            pv_chunk(0, 0)
            scores_chunk(1, 0)
            pv_chunk(0, 1)
            scores_chunk(1, 1)
            pv_chunk(1, 0)
            proj_chunk(0)
            scores_chunk(2, 0)
            pv_chunk(1, 1)
            scores_chunk(2, 1)
            pv_chunk(2, 0)
            proj_chunk(1)
            scores_chunk(3, 0)
            pv_chunk(2, 1)
            scores_chunk(3, 1)
            pv_chunk(3, 0)
            proj_chunk(2)
            pv_chunk(3, 1)
            proj_chunk(3)

    nc.finalize()
    return nc


def make_core_inputs(inputs, core, with_bias, _cache={}):
    """Host-side shard prep for one core."""
    b, g = core // 4, core % 4
    key = id(inputs)
    if _cache.get("key") != key:
        _cache.clear()
        _cache["key"] = key

    if ("hT", b) not in _cache:
        hidden = np.asarray(inputs["hidden_states"], dtype=np.float32)
        _cache[("hT", b)] = np.ascontiguousarray(hidden[b].T).astype(BF)
    if ("trig", b) not in _cache:
        pos = np.asarray(inputs["position_ids"])
        inv_freq = (1.0 / (10000.0 **
                           (np.arange(0, 64, 2, dtype=np.float64) / 64.0)))
        # pattern[d, s] = pos[s] * invf[d % 32] over d in [0, 64)
        freqsT = inv_freq[:, None] * pos[b].astype(np.float64)[None, :]
        embT = np.concatenate([freqsT, freqsT], axis=0)     # [64, S]
        cosp = np.cos(embT)
        sinp = np.sin(embT)
        sinp[:32, :] *= -1.0
        _cache[("trig", b)] = (np.tile(cosp, (2, 1)).astype(BF),
                               np.tile(sinp, (2, 1)).astype(BF))

    caw = np.asarray(inputs["c_attn_w"], dtype=np.float32)
    cab = np.asarray(inputs["c_attn_b"], dtype=np.float32)
    cpw = np.asarray(inputs["c_proj_w"], dtype=np.float32)
    cpb = np.asarray(inputs["c_proj_b"], dtype=np.float32)

    cs = slice(g * HD, (g + 1) * HD)
    wqkv = np.concatenate(
        [caw[:, cs], caw[:, D + g * HD:D + (g + 1) * HD],
         caw[:, 2 * D + g * HD:2 * D + (g + 1) * HD]], axis=1)

    bp = (cpb if g == 0 else np.zeros_like(cpb)).reshape(8, 128).T.copy()

    r = np.arange(128)
    mask01 = (r[None, :] >= r[:, None]).astype(BF)
    cosT, sinT = _cache[("trig", b)]

    out = {
        "hT": _cache[("hT", b)],
        "wqkv": np.ascontiguousarray(wqkv).astype(BF),
        "cosT": cosT,
        "sinT": sinT,
        "wp": np.ascontiguousarray(cpw[cs, :]).astype(BF),
        "bp": np.ascontiguousarray(bp.astype(np.float32)),
        "mask01": mask01,
        "ones64": np.ones((128, 64), BF),
        "ident": np.eye(128).astype(BF),
    }
    if with_bias:
        bqkv = np.concatenate(
            [cab[cs], cab[D + g * HD:D + (g + 1) * HD],
             cab[2 * D + g * HD:2 * D + (g + 1) * HD]])[None, :]
        out["bqkv"] = bqkv.astype(BF)
        out["ones_row"] = np.ones((1, 512), BF)
    return out


_NC_CACHE = {}


def run(inputs, trace=False, **spmd_kwargs):
    """Shard, execute on 8 cores, unshard. Returns (output, BassKernelResults)."""
    with_bias = bool(np.any(np.asarray(inputs["c_attn_b"])) or
                     np.any(np.asarray(inputs["c_proj_b"])))
    if with_bias not in _NC_CACHE:
        _NC_CACHE[with_bias] = build_attention_nc(with_bias=with_bias,
                                                  num_devices=8)
    nc = _NC_CACHE[with_bias]
    in_maps = [make_core_inputs(inputs, c, with_bias) for c in range(8)]
    res = run_bass_kernel_spmd(nc, in_maps, core_ids=list(range(8)),
                               trace=trace, **spmd_kwargs)
    outs = []
    for b in range(2):
        acc = np.zeros((D, S), np.float32)
        for g in range(4):
            acc += res.results[b * 4 + g]["outT"].astype(np.float32)
        outs.append(acc.T)
    return np.stack(outs, axis=0), res


def kernel(**inputs) -> np.ndarray:
    out, _ = run(inputs, trace=False)
    return out
